# revision 19
# baseline (speedup 1.0000x reference)
"""Trainium2 Bass kernel for nn_CGP_8899172237465 (gnn_message_passing).

The whole network is linear in x:
  - GATENet + degree-norm produce a 62x62 matrix A_norm from tiny inputs.
  - Each Euler step is cur <- (0.75 I + 0.25 A_norm) @ cur  == M @ cur.
  - states = [x, Mx, M^2 x, M^3 x, M^4 x]; 1x1 conv mixes the 5*32=160
    channels with mlp_w ([32,160]) and adds mlp_b.

So out[n,o,v,l] = sum_{t,c} P_t[o,c] * (M^t)[v,w] * x[n,c,w,l] + b[o]
               = W_big[(o,v),(c,w)] @ x[n,(c,w),l] + b[o],
with W_big = sum_t kron(P_t, M^t)  (1984x1984, precomputed on host --
it only depends on the tiny adjacency/weight inputs).

Device kernel (per core, data-parallel over batch: 8 cores x 4 batches):
tiled 2048x2048 (zero-padded) @ 2048x512 matmul in fp16 (weights get the
fast-weight-load path; 128-row K-tiles / 128-col M-tiles), fp32 PSUM
accumulation, bias added via ScalarE activation, DMA in/out.
Set NN_KERNEL_F32R=1 for the slower but more accurate fp32r variant.
"""

import os

import numpy as np

V = 62
B, C, L = 32, 32, 512
NST = 5            # states 0..4
KK = C * V         # 1984
N_CORES = 8
BPC = B // N_CORES  # batches per core = 4

USE_F32R = bool(os.environ.get("NN_KERNEL_F32R"))
T = 124 if USE_F32R else 128   # K/M tile size
KP = T * 16                    # padded operator dim (1984 or 2048)

_CACHE = {}


def _host_operator(adj_PLI, adj_buf, gate_w1, gate_w2, mlp_w, mlp_b):
    """W_big^T in the SBUF weight layout + bias layout (host, fp64)."""
    a64 = lambda a: np.asarray(a, dtype=np.float64)
    adj_PLI, adj_buf = a64(adj_PLI), a64(adj_buf)
    gate_w1, gate_w2, mlp_w, mlp_b = a64(gate_w1), a64(gate_w2), a64(mlp_w), a64(mlp_b)

    y = adj_buf @ gate_w1.T
    y = np.where(y > 0, y, np.expm1(y))          # ELU
    y = y @ gate_w2.T
    y = np.maximum(np.tanh(y), 0.0)              # ReLU(Tanh)
    adj_ds = y.reshape(V, V)
    adj = adj_PLI @ adj_ds + np.eye(V)
    d_inv = adj.sum(1) ** -0.5
    adj_norm = d_inv[:, None] * adj * d_inv[None, :]
    M = 0.75 * np.eye(V) + 0.25 * adj_norm

    Mp = [np.eye(V)]
    for _ in range(NST - 1):
        Mp.append(M @ Mp[-1])

    W = np.zeros((KP, KP))
    Wv = W[:KK, :KK].reshape(C, V, C, V)
    for t in range(NST):
        P_t = mlp_w[:, t * C:(t + 1) * C]        # [o, c]
        Wv += P_t[:, None, :, None] * Mp[t][None, :, None, :]

    WT = np.ascontiguousarray(W.T)               # [(c,w) K-padded, (o,v) M-padded]
    # SBUF layout: [p=T, (j,k,m)] with WT[T*k+p, T*j+m] -- j-major so each
    # output-column block is one contiguous DMA
    w_sb = WT.reshape(16, T, 16, T).transpose(1, 2, 0, 3).reshape(T, 16 * 16 * T)
    # bias per output row (o,v): b[o]; zero on padded rows
    rows = (np.arange(16)[None, :] * T + np.arange(T)[:, None])  # [p, j]
    bias_sb = np.where(rows < KK, mlp_b[np.minimum(rows // V, C - 1)], 0.0)
    w_dt = np.float32 if USE_F32R else np.float16
    w_np = np.ascontiguousarray(w_sb, dtype=w_dt)
    return w_np, np.ascontiguousarray(bias_sb, dtype=np.float32)


def _build_program(reps=1):
    from contextlib import ExitStack
    from concourse import bacc, tile, mybir

    nc = bacc.Bacc("TRN2", target_bir_lowering=False, debug=False,
                   enable_asserts=True, num_devices=N_CORES)
    f32 = mybir.dt.float32
    mdt = mybir.dt.float32r if USE_F32R else mybir.dt.float16

    x_ap = nc.dram_tensor("x", [BPC, KP, L], mdt, kind="ExternalInput").ap()
    w_ap = nc.dram_tensor("wt", [T, 16 * 16 * T], mdt, kind="ExternalInput").ap()
    b_ap = nc.dram_tensor("bias", [T, 16], f32, kind="ExternalInput").ap()
    o_ap = nc.dram_tensor("out", [BPC, KP, L], f32, kind="ExternalOutput").ap()

    with tile.TileContext(nc) as tc, ExitStack() as ctx:
        wpool = ctx.enter_context(tc.tile_pool(name="w", bufs=1))
        xpool = ctx.enter_context(tc.tile_pool(name="x", bufs=24))
        opool = ctx.enter_context(tc.tile_pool(name="o", bufs=8))
        pspool = ctx.enter_context(tc.tile_pool(name="ps", bufs=8, space="PSUM"))

        b_sb = wpool.tile([T, 16], f32)
        nc.sync.dma_start(b_sb[:], b_ap[:])
        # W split into 16 j-column tiles so the first matmuls only wait on
        # the first 1/16th of the 8.4MB weight load
        w_js = []
        for j in range(16):
            w_j = wpool.tile([T, 16 * T], mdt, name=f"w{j}")
            nc.sync.dma_start(w_j[:], w_ap[:, j * 16 * T:(j + 1) * 16 * T])
            w_js.append(w_j)

        def body():
            # n-outer with just-in-time x loads: batch n+1's DMAs overlap
            # batch n's matmuls (xk pool has slack beyond one batch).
            for n in range(BPC):
                xt = []
                for k in range(16):
                    t_x = xpool.tile([T, L], mdt, name=f"x{n}_{k}", tag="xk")
                    nc.sync.dma_start(t_x[:], x_ap[n, k * T:(k + 1) * T, :])
                    xt.append(t_x)
                for j in range(16):
                    ps = pspool.tile([T, L], f32, name="ps", tag="ps")
                    for k in range(16):
                        wsl = w_js[j][:, k * T:(k + 1) * T]
                        nc.tensor.matmul(ps[:], wsl, xt[k][:],
                                         start=(k == 0), stop=(k == 15))
                    ob = opool.tile([T, L], f32, name="ob", tag="ob")
                    nc.scalar.activation(ob[:], ps[:],
                                         mybir.ActivationFunctionType.Identity,
                                         bias=b_sb[:, j:j + 1])
                    nc.sync.dma_start(o_ap[n, j * T:(j + 1) * T, :], ob[:])

        if reps == 1:
            body()
        else:
            with tc.For_i(0, reps, 1):
                body()

    nc.compile()
    return nc


def _prep_x(x):
    """[B, C, V, L] fp32 -> padded [B, KP, L] in the matmul dtype."""
    xr = np.asarray(x, dtype=np.float32).reshape(B, KK, L)
    xp = np.zeros((B, KP, L), dtype=np.float32 if USE_F32R else np.float16)
    xp[:, :KK] = xr
    return xp


def kernel(x, adj_PLI, adj_buf, gate_w1, gate_w2, mlp_w, mlp_b):
    from concourse.bass_utils import run_bass_kernel_spmd

    w_np, bias_sb = _host_operator(adj_PLI, adj_buf, gate_w1, gate_w2,
                                   mlp_w, mlp_b)
    xp = _prep_x(x)

    if "nc" not in _CACHE:
        _CACHE["nc"] = _build_program()
    nc = _CACHE["nc"]

    in_maps = [
        {"x": np.ascontiguousarray(xp[i * BPC:(i + 1) * BPC]),
         "wt": w_np, "bias": bias_sb}
        for i in range(N_CORES)
    ]
    res = run_bass_kernel_spmd(nc, in_maps, list(range(N_CORES)))
    if res.exec_time_ns is not None:
        print(f"HW exec time: {res.exec_time_ns} ns")
        _CACHE["exec_time_ns"] = res.exec_time_ns

    out = np.empty((B, C, V, L), dtype=np.float32)
    for i in range(N_CORES):
        out[i * BPC:(i + 1) * BPC] = \
            res.results[i]["out"][:, :KK].reshape(BPC, C, V, L)
    return out
